# revision 2
# baseline (speedup 1.0000x reference)
"""Complex multi-head attention on 8 Trainium2 NeuronCores — v2.

Same sharding as v1: core c handles batch b = c//2 and heads
4*(c%2)..4*(c%2)+4. Optimized for the observed per-instruction-dominated
execution cost: fewer, larger instructions.

Changes vs v1:
  - all inputs packed host-side into 5 DRAM tensors -> 5 input DMAs
    (was ~45 small DMAs)
  - scores PSUM tiles are [128, 1024] (two banks); the square/sqrt/exp
    chain runs on full [128,1024] tiles: 5 elementwise ops per (h,kc)
    instead of 8
  - softmax scale folded into the Q projection bias/scale (host-side)
  - Z computed by accumulating e over kc in SBUF (vector adds) then two
    ones-matmuls per head (was 16 ones-matmuls per head)
  - one reciprocal per head on [1,1024] (was 2x [1,512])
"""

import numpy as np

B, S, E, H = 4, 1024, 512, 8
HD = E // H  # 64
SCALE = HD ** -0.5
N_CORES = 8
HPC = H // 2          # heads per core = 4
FPC = HPC * HD        # feature cols per core = 256
EC = E // 128         # contraction chunks = 4
QB = S // 512         # 512-wide q blocks = 2
KC = S // 128         # 128-wide k chunks = 8
SC = S // 128         # s chunks for V = 8

_CACHE = {}


def _patch_tile_drain():
    """The tile-exit drain attaches one sem wait per live logical
    processor; this walrus build accepts a single sync wait per CTRL
    instruction. Split the waits across a chain of drains on the same
    engine (program order preserves the semantics)."""
    import concourse.tile as tile_mod
    from concourse.vector_clock import ScopedClock

    if getattr(tile_mod.TileContext, "_drain_split_patched", False):
        return

    def _patched(self, tick_clock, wait_clock):
        nc = self.nc
        drain_inst = nc.sync.drain()
        wait_clock.add_sem_waits(
            drain_inst.ins, ScopedClock({None: tick_clock.global_clock})
        )
        si = drain_inst.ins.sync_info
        waits = list(si.on_wait) if si and si.on_wait else []
        if len(waits) > 1:
            si.on_wait = waits[:1]
            for w in waits[1:]:
                extra = nc.sync.drain()
                esi = extra.ins.sync_info
                if esi is None:
                    import concourse.mybir as mybir
                    extra.ins.sync_info = mybir.SyncInfo(on_wait=[w], on_update=[])
                else:
                    esi.on_wait = list(esi.on_wait or []) + [w]
        nc.all_engine_barrier()
        assert self.sems is not None
        popped = nc._tile_sem_poison_stack.pop()
        assert popped is self._sem_poison
        nc.clear_and_free_semaphores(list(self.sems.allocated().values()))
        nc.all_engine_barrier()

    tile_mod.TileContext._drain_and_barrier = _patched
    tile_mod.TileContext._drain_split_patched = True


def _split_multi_waits(nc):
    """This walrus build accepts a single sync wait per instruction.
    Hoist extra waits onto same-engine NOPs inserted just before the
    instruction (waits execute earlier on the same engine: semantics
    preserved, strictly more conservative)."""
    import concourse.mybir as mybir

    ctr = [0]
    for f in nc.m.functions:
        for bb in f.blocks:
            out = []
            changed = False
            for ins in bb.instructions:
                si = ins.sync_info
                waits = list(si.on_wait) if si and si.on_wait else []
                if len(waits) > 1:
                    changed = True
                    for w in waits[:-1]:
                        ctr[0] += 1
                        nop = mybir.InstNoOp(
                            name=f"W-split-{ctr[0]}",
                            sync_info=mybir.SyncInfo(on_wait=[w], on_update=[]),
                            engine=ins.engine,
                            bass_nofuse=True,
                        )
                        out.append(nop)
                    si.on_wait = waits[-1:]
                out.append(ins)
            if changed:
                bb.instructions = out
    return ctr[0]


def _serialize_sync(nc):
    """Fully serialize the body on the engine tick semaphores: each
    instruction waits only on its global predecessor (omitted when on the
    same engine — program order implies it). The backend executes
    instructions with no cross-engine overlap anyway, so this costs no
    parallelism and eliminates every multi-wait NoOp split.

    Strictly more conservative than the Tile-derived dependencies, so
    race-free by construction. Engine tick updates are left untouched
    (the closing drains wait on their final values)."""
    from concourse import mybir

    fn = nc.m.functions[0]
    bb = max(fn.blocks, key=lambda b: len(b.instructions))
    tick = {}
    prev = None  # (sem_id, ant_name, value_after)
    for ins in bb.instructions:
        si = ins.sync_info
        ups = list(si.on_update) if si and si.on_update else []
        if not ups:
            continue  # end-of-block branches
        u = ups[0]
        if prev is not None and prev[0] != u.id:
            si.on_wait = [mybir.SyncWait(
                sync_type="semaphore", id=prev[0], ant_name=prev[1],
                wait_mode="sem-ge-imm", wait_value=prev[2])]
        else:
            si.on_wait = []
        inc = u.update_value if u.update_mode in ("sem-inc", "sem-add-imm") else 1
        v = tick.get(u.id, 0) + inc
        tick[u.id] = v
        prev = (u.id, u.ant_name, v)


# win column offsets per stack (qa, qb, ka, kb, va, vb), each 4*512 wide
_W_OFF = {n: i * 2048 for i, n in enumerate(("qa", "qb", "ka", "kb", "va", "vb"))}


def _build_program(reps=1):
    import concourse.bass as bass
    from concourse import mybir
    from concourse.tile import TileContext
    from contextlib import ExitStack

    _patch_tile_drain()

    f32 = mybir.dt.float32
    f32r = mybir.dt.float32r

    nc = bass.Bass()
    dp = nc.declare_dram_parameter
    xin = dp("xin", [128, 8192], f32r, isOutput=False)    # xr 4x1024 | xi 4x1024
    win = dp("win", [128, 12288], f32r, isOutput=False)   # 6 stacks x (4ec x 512)
    wod = dp("wod", [128, 4096], f32r, isOutput=False)    # wor 4h x 512 | woi 4h x 512
    cst = dp("cst", [128, 8], f32, isOutput=False)        # bq(SCALEd) 0:4 | bk 4:8
    ones_d = dp("ones", [128, 1], f32r, isOutput=False)
    yrT = dp("yrT", [E, S], f32, isOutput=True)
    yiT = dp("yiT", [E, S], f32, isOutput=True)

    Exp = mybir.ActivationFunctionType.Exp
    Sqrt = mybir.ActivationFunctionType.Sqrt
    Square = mybir.ActivationFunctionType.Square

    with TileContext(nc) as tc:
        for _rep in range(reps):
          with ExitStack() as outer:
            consts = outer.enter_context(tc.tile_pool(name="consts", bufs=1))
            qkv_out = outer.enter_context(tc.tile_pool(name="qkv", bufs=1))
            avs = outer.enter_context(tc.tile_pool(name="avs", bufs=1))
            wo_pool = outer.enter_context(tc.tile_pool(name="wo", bufs=1))
            yout = outer.enter_context(tc.tile_pool(name="yout", bufs=2))

            ones_sb = consts.tile([128, 1], f32r)
            nc.sync.dma_start(out=ones_sb, in_=ones_d[:, :])
            cst_sb = consts.tile([128, 8], f32)
            nc.sync.dma_start(out=cst_sb, in_=cst[:, :])
            wo_sb = wo_pool.tile([128, 4096], f32r)
            nc.sync.dma_start(out=wo_sb, in_=wod[:, :])

            # persistent attention operands
            Q_sb = [qkv_out.tile([128, S], f32r, tag=f"Q{h}", name=f"Q{h}") for h in range(HPC)]
            K_sb = [qkv_out.tile([128, S], f32r, tag=f"K{h}", name=f"K{h}") for h in range(HPC)]
            K2_sb = [qkv_out.tile([128, S], f32r, tag=f"K2{h}", name=f"K2{h}") for h in range(HPC)]
            V_sb = [qkv_out.tile([128, FPC * 2], f32r, tag=f"V{sc}", name=f"V{sc}") for sc in range(SC)]
            av_sb = [avs.tile([128, S], f32r, tag=f"av{h}", name=f"av{h}") for h in range(HPC)]

            # ---------------- QKV projections ----------------
            with ExitStack() as qkv_ctx:
                xw = qkv_ctx.enter_context(tc.tile_pool(name="xw", bufs=1))
                psum_p = qkv_ctx.enter_context(
                    tc.tile_pool(name="psum_p", bufs=2, space="PSUM")
                )

                x_sb = xw.tile([128, 8192], f32r)
                nc.sync.dma_start(out=x_sb, in_=xin[:, :])
                w_sb = xw.tile([128, 12288], f32r)
                nc.sync.dma_start(out=w_sb, in_=win[:, :])

                def xr(ec, qs):  # [128, 512] slice of xr chunk ec
                    return x_sb[:, 1024 * ec + qs.start:1024 * ec + qs.stop]

                def xi(ec, qs):
                    return x_sb[:, 4096 + 1024 * ec + qs.start:4096 + 1024 * ec + qs.stop]

                def w(name, ec, cs):  # [128, len(cs)] slice of weight stack
                    base = _W_OFF[name] + 512 * ec
                    return w_sb[:, base + cs.start:base + cs.stop]

                # Q and K (transposed layout, bias per partition)
                for h in range(HPC):
                    cs = slice(128 * h, 128 * h + 128)
                    for dst, wa, wb, bcol in (
                        (Q_sb[h], "qa", "qb", h),
                        (K_sb[h], "ka", "kb", 4 + h),
                    ):
                        ps = psum_p.tile([128, S], f32, tag="proj_ps", name="proj_ps")
                        for qb in range(QB):
                            qs = slice(512 * qb, 512 * qb + 512)
                            for i, ec in enumerate(range(EC)):
                                nc.tensor.matmul(
                                    ps[:, qs], w(wa, ec, cs), xr(ec, qs),
                                    start=(i == 0), stop=False)
                            for i, ec in enumerate(range(EC)):
                                nc.tensor.matmul(
                                    ps[:, qs], w(wb, ec, cs), xi(ec, qs),
                                    start=False, stop=(i == EC - 1))
                        if bcol < 4:  # Q: bias pre-scaled, also scale scores
                            nc.vector.tensor_scalar(
                                dst, ps, SCALE, cst_sb[:, bcol:bcol + 1],
                                mybir.AluOpType.mult, mybir.AluOpType.add)
                        else:
                            nc.vector.tensor_scalar_add(dst, ps, cst_sb[:, bcol:bcol + 1])
                    # K2 = [-ki; kr] from K (biases already included)
                    nc.vector.tensor_scalar_mul(K2_sb[h][0:64, :], K_sb[h][64:128, :], -1.0)
                    nc.scalar.copy(K2_sb[h][64:128, :], K_sb[h][0:64, :])

                # V natural layout (no bias; folded into host constants)
                for sc in range(SC):
                    ss = slice(128 * sc, 128 * sc + 128)
                    ps = psum_p.tile([128, FPC * 2], f32, tag="v_ps", name="v_ps")
                    for i, ec in enumerate(range(EC)):
                        nc.tensor.matmul(ps, xr(ec, ss), w("va", ec, slice(0, 512)),
                                         start=(i == 0), stop=False)
                    for i, ec in enumerate(range(EC)):
                        nc.tensor.matmul(ps, xi(ec, ss), w("vb", ec, slice(0, 512)),
                                         start=False, stop=(i == EC - 1))
                    nc.scalar.copy(V_sb[sc], ps)

            # ---------------- attention ----------------
            with ExitStack() as att_ctx:
                sc_pool = att_ctx.enter_context(
                    tc.tile_pool(name="sc_ps", bufs=1, space="PSUM"))
                av_pool = att_ctx.enter_context(
                    tc.tile_pool(name="av_ps", bufs=1, space="PSUM"))
                z_pool = att_ctx.enter_context(
                    tc.tile_pool(name="z_ps", bufs=1, space="PSUM"))
                ew = att_ctx.enter_context(tc.tile_pool(name="ew", bufs=2))
                epool = att_ctx.enter_context(tc.tile_pool(name="epool", bufs=3))
                eacc_p = att_ctx.enter_context(tc.tile_pool(name="eacc", bufs=2))
                zdram = att_ctx.enter_context(
                    tc.tile_pool(name="zdram", bufs=2, space="DRAM"))

                for h in range(HPC):
                    av_ps = [av_pool.tile([128, 512], f32, tag=f"avp{qb}", name=f"avp{qb}")
                             for qb in range(QB)]
                    e_acc = eacc_p.tile([128, S], f32r, tag="eacc", name="eacc")
                    for kc in range(KC):
                        ks = slice(128 * kc, 128 * kc + 128)
                        # one 4-bank PSUM tile: sr in [0:1024], si in [1024:2048]
                        sps = sc_pool.tile([128, 2 * S], f32, tag="s", name="s")
                        for qb in range(QB):
                            qs = slice(512 * qb, 512 * qb + 512)
                            qs2 = slice(S + 512 * qb, S + 512 * qb + 512)
                            nc.tensor.matmul(sps[:, qs], K_sb[h][:, ks], Q_sb[h][:, qs],
                                             start=True, stop=True)
                            nc.tensor.matmul(sps[:, qs2], K2_sb[h][:, ks], Q_sb[h][:, qs],
                                             start=True, stop=True)
                        u = ew.tile([128, 2 * S], f32, tag="u", name="u")
                        wt = ew.tile([128, S], f32, tag="w", name="w")
                        nc.scalar.activation(u, sps, Square)
                        nc.gpsimd.tensor_tensor(wt, u[:, 0:S], u[:, S:2 * S],
                                                mybir.AluOpType.add)
                        nc.scalar.activation(u[:, 0:S], wt, Sqrt)
                        e_t = epool.tile([128, S], f32r, tag="e", name="e")
                        nc.scalar.activation(e_t, u[:, 0:S], Exp)

                        for qb in range(QB):
                            qs = slice(512 * qb, 512 * qb + 512)
                            nc.tensor.matmul(
                                av_ps[qb], V_sb[kc][:, 128 * h:128 * h + 128],
                                e_t[:, qs], start=(kc == 0), stop=(kc == KC - 1))
                        if kc == 0:
                            nc.vector.tensor_copy(e_acc, e_t)
                        else:
                            nc.vector.tensor_add(e_acc, e_acc, e_t)

                    # Z = ones^T e_acc; zr = 1/Z; broadcast via DRAM round trip
                    z_ps = z_pool.tile([1, S], f32, tag="zp", name="zp")
                    for qb in range(QB):
                        qs = slice(512 * qb, 512 * qb + 512)
                        nc.tensor.matmul(z_ps[:, qs], ones_sb, e_acc[:, qs],
                                         start=True, stop=True)
                    zr = ew.tile([1, S], f32, tag="zr", name="zr")
                    nc.vector.reciprocal(zr, z_ps)
                    zd = zdram.tile([1, S], f32, tag="zd", name="zd")
                    nc.sync.dma_start(out=zd, in_=zr)
                    zb = ew.tile([128, S], f32, tag="zb", name="zb")
                    zd_b = bass.AP(
                        tensor=zd.tensor, offset=zd.offset,
                        ap=[[0, 128]] + list(zd.ap[1:]))
                    nc.sync.dma_start(out=zb, in_=zd_b)
                    for qb in range(QB):
                        qs = slice(512 * qb, 512 * qb + 512)
                        nc.vector.tensor_mul(av_sb[h][:, qs], av_ps[qb], zb[:, qs])

            # ---------------- output projection: yrT/yiT [E, S] ----------------
            with ExitStack() as yctx:
                y_psum = yctx.enter_context(
                    tc.tile_pool(name="y_ps", bufs=2, space="PSUM"))
                for di, dst in enumerate((yrT, yiT)):
                    for eco in range(EC):
                        es = slice(128 * eco, 128 * eco + 128)
                        ps = y_psum.tile([128, S], f32, tag="y", name="y")
                        for qb in range(QB):
                            qs = slice(512 * qb, 512 * qb + 512)
                            for h in range(HPC):
                                wslice = wo_sb[:, 2048 * di + 512 * h + es.start:
                                               2048 * di + 512 * h + es.stop]
                                nc.tensor.matmul(
                                    ps[:, qs], wslice, av_sb[h][:, qs],
                                    start=(h == 0), stop=(h == HPC - 1))
                        yt = yout.tile([128, S], f32, tag="yt", name="yt")
                        nc.scalar.copy(yt, ps)
                        nc.sync.dma_start(out=dst[es, :], in_=yt)

    _serialize_sync(nc)
    _split_multi_waits(nc)
    return nc


def _prep_core_inputs(inputs, c):
    f32 = np.float32
    b, j = c // 2, c % 2
    hs = slice(FPC * j, FPC * j + FPC)

    def stacks(wr, wi):
        # A (applied to xr^T) and B (applied to xi^T): per head h the
        # 128-col block is [wr[fs].T | wi[fs].T] resp. [-wi[fs].T | wr[fs].T]
        A = np.empty((E, FPC * 2), f32)
        Bm = np.empty((E, FPC * 2), f32)
        for h in range(HPC):
            fs = slice(hs.start + HD * h, hs.start + HD * h + HD)
            A[:, 128 * h:128 * h + 64] = wr[fs, :].T
            A[:, 128 * h + 64:128 * h + 128] = wi[fs, :].T
            Bm[:, 128 * h:128 * h + 64] = -wi[fs, :].T
            Bm[:, 128 * h + 64:128 * h + 128] = wr[fs, :].T
        return A, Bm

    wqa, wqb = stacks(inputs["q_wr"], inputs["q_wi"])
    wka, wkb = stacks(inputs["k_wr"], inputs["k_wi"])
    wva, wvb = stacks(inputs["v_wr"], inputs["v_wi"])

    def pack_stack(Astk):  # [E, 512] -> [128, 2048] (ec-major)
        out = np.empty((128, EC * 512), f32)
        for ec in range(EC):
            out[:, 512 * ec:512 * ec + 512] = Astk[128 * ec:128 * ec + 128, :]
        return out

    win = np.concatenate(
        [pack_stack(a) for a in (wqa, wqb, wka, wkb, wva, wvb)], axis=1)

    def bias_cols(br, bi):
        out = np.empty((128, HPC), f32)
        for h in range(HPC):
            fs = slice(hs.start + HD * h, hs.start + HD * h + HD)
            out[0:64, h] = br[fs] - bi[fs]
            out[64:128, h] = br[fs] + bi[fs]
        return out

    cst = np.empty((128, 8), f32)
    cst[:, 0:4] = bias_cols(inputs["q_br"], inputs["q_bi"]) * SCALE
    cst[:, 4:8] = bias_cols(inputs["k_br"], inputs["k_bi"])

    o_wr, o_wi = inputs["o_wr"], inputs["o_wi"]
    wod = np.empty((128, 4096), f32)
    for h in range(HPC):
        fs = slice(hs.start + HD * h, hs.start + HD * h + HD)
        wod[0:64, 512 * h:512 * h + 512] = o_wr[:, fs].T
        wod[64:128, 512 * h:512 * h + 512] = -o_wi[:, fs].T
        wod[0:64, 2048 + 512 * h:2048 + 512 * h + 512] = o_wi[:, fs].T
        wod[64:128, 2048 + 512 * h:2048 + 512 * h + 512] = o_wr[:, fs].T

    xrT = inputs["x_real"][b].T.astype(f32)
    xiT = inputs["x_imag"][b].T.astype(f32)
    xin = np.empty((128, 8192), f32)
    for ec in range(EC):
        xin[:, 1024 * ec:1024 * ec + 1024] = xrT[128 * ec:128 * ec + 128, :]
        xin[:, 4096 + 1024 * ec:4096 + 1024 * ec + 1024] = xiT[128 * ec:128 * ec + 128, :]

    return {
        "xin": np.ascontiguousarray(xin),
        "win": np.ascontiguousarray(win),
        "wod": np.ascontiguousarray(wod),
        "cst": np.ascontiguousarray(cst),
        "ones": np.ones((128, 1), f32),
    }


def kernel(**inputs):
    from concourse.bass_utils import run_bass_kernel_spmd

    if "nc" not in _CACHE:
        _CACHE["nc"] = _build_program()
    nc = _CACHE["nc"]

    core_ids = list(range(N_CORES))
    in_maps = [_prep_core_inputs(inputs, c) for c in core_ids]
    res = run_bass_kernel_spmd(nc, in_maps, core_ids)

    # host-side unshard: sum partial yT pairs, add bias constants, transpose
    f32 = np.float32
    o_wr, o_wi = inputs["o_wr"], inputs["o_wi"]
    cvr = inputs["v_br"] - inputs["v_bi"]
    cvi = inputs["v_br"] + inputs["v_bi"]
    yr_const = (inputs["o_br"] - inputs["o_bi"]) + o_wr @ cvr - o_wi @ cvi
    yi_const = (inputs["o_br"] + inputs["o_bi"]) + o_wi @ cvr + o_wr @ cvi

    yr = np.empty((B, S, E), f32)
    yi = np.empty((B, S, E), f32)
    for b in range(B):
        r0, r1 = res.results[2 * b], res.results[2 * b + 1]
        yr[b] = (r0["yrT"] + r1["yrT"]).T + yr_const[None, :]
        yi[b] = (r0["yiT"] + r1["yiT"]).T + yi_const[None, :]
    return yr, yi
